# revision 6
# baseline (speedup 1.0000x reference)
"""Trainium2 Bass kernel for nn_CausalBiBCNAttention (B=4, T=4096, D=1024, R=256).

Algebra (exact rewrite of the reference):
    out = G @ (Wo@U).T + min(n,1)*(1+alpha)*(Wo@bias)
    G   = (A*cumsum(Bk) + E*cumsum(C)) / max(n,1)
    A   = x @ (Wq.T V);  E = x @ (Wq.T Winv.T Wm)
    Bk  = (x @ (Wk.T Wm)) * m;  C = alpha * (x @ (Wk.T Winv.T V)) * m
    n   = cumsum(m)
The five DxD projections fold into four DxR matrices (host constant folding in
f64). Host-side prep folds the row scalings into the x streams:
    xs = x * (1/max(n,1))   (A/E stream -> G's division by n comes for free)
    xk = x * m              (K stream   -> masking comes for free)
so the device does only: 8 rank-128 projection groups per 512-col chunk,
native DVE prefix scans (f32 state, f16 out), two f16 multiplies + add for G,
and the final rank-R contraction with (Wo U).T. Everything streams fp16
single-pass (the 2e-2 harness gate leaves plenty of margin; measured ~5e-4).

All tensors are staged host-side in the exact SBUF image layout (2D
contiguous, 128 descriptors/DMA) and the transfers are spread across the
three DMA queues (SP + ACT hardware DGE, Pool software DGE) so input
streaming never gates the PE.

Sharding: 8 cores = batch(4) x sequence-halves(2). The cumsum carry S for the
second half and the 1/n rows are computed on the host (cheap O(B*T*D) numpy)
and passed as tiny inputs, so no cross-core or xprev streaming is needed.
"""

from contextlib import ExitStack

import numpy as np

import concourse.bass as bass
import concourse.mybir as mybir
import concourse.tile as tile
from concourse.bass_utils import run_bass_kernel_spmd

F32 = mybir.dt.float32
F16 = mybir.dt.float16
AL = mybir.AluOpType

N_CORES = 8
N_SEQ_SHARDS = 2


def fold_weights(Wq, Wk, Wo, Winv, U, V, Wm, bias, alpha):
    Wq, Wk, Wo, Winv, U, V, Wm, bias = (
        np.asarray(a, np.float64) for a in (Wq, Wk, Wo, Winv, U, V, Wm, bias)
    )
    alpha = float(alpha)
    P1 = Wq.T @ V
    P2 = Wq.T @ Winv.T @ Wm
    P3 = Wk.T @ Wm
    P4 = alpha * (Wk.T @ (Winv.T @ V))
    PAE = np.concatenate([P1, P2], axis=1)          # [D, 2R] f64
    PK = np.concatenate([P3, P4], axis=1)           # [D, 2R] f64
    ZT = np.ascontiguousarray((Wo @ U).T)           # [R, D] f64
    bvec = ((1.0 + alpha) * (Wo @ bias))            # [D] f64
    return PAE, PK, ZT, bvec


def split_excess_waits(nc, max_waits=1):
    """Hoist excess per-instruction sync waits onto preceding same-engine NoOps.

    Walrus's per-instruction sync budget rejects >1 wait command on several
    instruction structs (fp32 Matmult, DMA pseudo-ops). Engine streams execute
    in order, so a NoOp carrying the extra wait immediately before the
    instruction is semantically identical.
    """
    fn = nc.m.functions[0]
    k = 0
    for blk in fn.blocks:
        new_insts = []
        for ins in blk.instructions:
            si = getattr(ins, "sync_info", None)
            if si is not None and si.on_wait and len(si.on_wait) > max_waits:
                waits = list(si.on_wait)
                for w in waits[:-max_waits]:
                    k += 1
                    new_insts.append(
                        mybir.InstNoOp(
                            name=f"{ins.name}-hoistw{k}",
                            engine=ins.engine,
                            ins=[],
                            outs=[],
                            sync_info=mybir.SyncInfo(on_wait=[w], on_update=[]),
                            bass_nofuse=True,
                        )
                    )
                ins.sync_info = mybir.SyncInfo(
                    on_wait=waits[-max_waits:], on_update=si.on_update
                )
            new_insts.append(ins)
        blk.instructions[:] = new_insts
    return nc


def build_nc(D, TC, R, TT=512, with_bias=False, hoist=True):
    assert D % 128 == 0 and R % 128 == 0 and TC % TT == 0
    nd, nr, nt = D // 128, R // 128, TC // TT
    nq = 2 * nr            # cumsum streams: [Bk ranks | C ranks]
    W2 = 2 * R             # projection width per stream pair
    XW = nd * TT           # x-stream image columns per t-chunk

    nc = bass.Bass()
    # all inputs are pre-staged SBUF images: [128, cols], plain 2D DMAs
    xsD = nc.dram_tensor("xsD", (128, nt * XW), F16, kind="ExternalInput")
    xkD = nc.dram_tensor("xkD", (128, nt * XW), F16, kind="ExternalInput")
    PAEd = nc.dram_tensor("PAEd", (128, nd * W2), F16, kind="ExternalInput")
    PKd = nc.dram_tensor("PKd", (128, nd * W2), F16, kind="ExternalInput")
    ZTd = nc.dram_tensor("ZTd", (128, nr * D), F16, kind="ExternalInput")
    initd = nc.dram_tensor("initd", (128, nq), F32, kind="ExternalInput")
    if with_bias:
        minnd = nc.dram_tensor("minnd", (1, TC), F16, kind="ExternalInput")
        bvd = nc.dram_tensor("bvd", (1, D), F16, kind="ExternalInput")
    outD = nc.dram_tensor("outD", (128, nt * XW), F16, kind="ExternalOutput")

    with tile.TileContext(nc) as tc, ExitStack() as ctx:
        res = ctx.enter_context(tc.tile_pool(name="res", bufs=1))
        psb = ctx.enter_context(tc.tile_pool(name="psb", bufs=8, space="PSUM"))
        aep = ctx.enter_context(tc.tile_pool(name="aep", bufs=6))
        gwp = ctx.enter_context(tc.tile_pool(name="gwp", bufs=4))
        otp = ctx.enter_context(tc.tile_pool(name="otp", bufs=2))

        # resident tiles; x streams are t-major, d-minor column blocks
        xk = res.tile([128, nt * XW], F16, tag="xk", name="xk")
        xs = res.tile([128, nt * XW], F16, tag="xs", name="xs")
        pk = res.tile([128, nd * W2], F16, tag="pk", name="pk")
        pae = res.tile([128, nd * W2], F16, tag="pae", name="pae")
        zt = res.tile([128, nr * D], F16, tag="zt", name="zt")
        cums = [
            res.tile([128, TC], F16, tag=f"cum{q}", name=f"cum{q}")
            for q in range(nq)
        ]
        ghs = [
            [
                res.tile([128, TT], F16, tag=f"gh{r}_{t}", name=f"gh{r}_{t}")
                for t in range(nt)
            ]
            for r in range(nr)
        ]
        initt = res.tile([128, nq], F32, tag="initt", name="initt")
        zdum = res.tile([128, TT], F16, tag="zdum", name="zdum")
        if with_bias:
            minnt = res.tile([1, TC], F16, tag="minnt", name="minnt")
            bvt = res.tile([1, D], F16, tag="bvt", name="bvt")

        nc.vector.memset(zdum[:, :], 0.0)

        # DMA queue assignment (3 queues). The two HWDGE queues (SP, ACT)
        # come up ~3us before the Pool SWDGE queue, so the critical first
        # tensors (pk, xk chunk 0) ride them, split in d-halves so the first
        # K matmuls can start on the first half. xs / later chunks / outputs
        # have slack and ride Pool.
        HW2 = nd * W2 // 2
        nc.scalar.dma_start(initt[:, :], initd[:, :])
        nc.sync.dma_start(pk[:, 0:HW2], PKd[:, 0:HW2])
        nc.scalar.dma_start(xk[:, 0 : XW // 2], xkD[:, 0 : XW // 2])
        nc.sync.dma_start(pk[:, HW2:], PKd[:, HW2:])
        nc.scalar.dma_start(xk[:, XW // 2 : XW], xkD[:, XW // 2 : XW])
        for t in range(1, nt):
            nc.sync.dma_start(xk[:, t * XW : (t + 1) * XW], xkD[:, t * XW : (t + 1) * XW])
        nc.scalar.dma_start(pae[:, :], PAEd[:, :])
        for t in range(nt):
            nc.gpsimd.dma_start(xs[:, t * XW : (t + 1) * XW], xsD[:, t * XW : (t + 1) * XW])
        nc.scalar.dma_start(zt[:, :], ZTd[:, :])
        if with_bias:
            nc.scalar.dma_start(minnt[:, :], minnd[:, :])
            nc.scalar.dma_start(bvt[:, :], bvd[:, :])

        def emit_final(t, last=False):
            tsl = slice(t * TT, (t + 1) * TT)
            ot = otp.tile([128, XW], F16, tag="ot", name="ot")
            for dd in range(nd):
                po = psb.tile([128, TT], F32, tag="pt", name="pt")
                for r in range(nr):
                    nc.tensor.matmul(
                        po[:, :],
                        zt[:, r * D + dd * 128 : r * D + (dd + 1) * 128],
                        ghs[r][t][:, :],
                        start=(r == 0),
                        stop=(r == nr - 1 and not with_bias),
                    )
                if with_bias:
                    nc.tensor.matmul(
                        po[:, :],
                        bvt[0:1, dd * 128 : (dd + 1) * 128],
                        minnt[0:1, tsl],
                        start=False,
                        stop=True,
                    )
                osl = slice(dd * TT, (dd + 1) * TT)
                # alternate evacuation engines: a single ACT can't drain PSUM
                # as fast as the PE fills it in the final phase
                if dd % 2 == 1:
                    nc.vector.tensor_copy(ot[:, osl], po[:, :])
                else:
                    nc.scalar.copy(ot[:, osl], po[:, :])
                if last and dd == nd // 2 - 1:
                    nc.scalar.dma_start(
                        outD[:, t * XW : t * XW + XW // 2], ot[:, 0 : XW // 2]
                    )
            if last:
                nc.sync.dma_start(
                    outD[:, t * XW + XW // 2 : (t + 1) * XW], ot[:, XW // 2 :]
                )
            else:
                nc.gpsimd.dma_start(outD[:, t * XW : (t + 1) * XW], ot[:, :])

        # A/E stream order pairs (A_r, E_r) adjacently so gh[r] can start as
        # soon as its two operands are evacuated
        m_order = []
        for r in range(nr):
            m_order += [r, nr + r]

        prev_t = None
        for t in range(nt):
            tsl = slice(t * TT, (t + 1) * TT)
            xoff = t * XW
            last = t == nt - 1
            # K-side projections -> prefix scans (f32 state, f16 out)
            for q in range(nq):
                pt = psb.tile([128, TT], F32, tag="pt", name="pt")
                for dd in range(nd):
                    nc.tensor.matmul(
                        pt[:, :],
                        pk[:, dd * W2 + q * 128 : dd * W2 + (q + 1) * 128],
                        xk[:, xoff + dd * TT : xoff + (dd + 1) * TT],
                        start=(dd == 0),
                        stop=(dd == nd - 1),
                    )
                init = initt[:, q : q + 1] if t == 0 else cums[q][:, t * TT - 1 : t * TT]
                nc.vector.tensor_tensor_scan(
                    cums[q][:, tsl], pt[:, :], zdum[:, :], init, AL.add, AL.bypass
                )
            # A/E projections (xs carries the 1/n row scaling)
            aes = [None] * nq
            for k, mi in enumerate(m_order):
                pa = psb.tile([128, TT], F32, tag="pt", name="pt")
                for dd in range(nd):
                    nc.tensor.matmul(
                        pa[:, :],
                        pae[:, dd * W2 + mi * 128 : dd * W2 + (mi + 1) * 128],
                        xs[:, xoff + dd * TT : xoff + (dd + 1) * TT],
                        start=(dd == 0),
                        stop=(dd == nd - 1),
                    )
                ae = aep.tile([128, TT], F16, tag="ae", name="ae")
                if last and k % 2 == 1:
                    nc.vector.tensor_copy(ae[:, :], pa[:, :])
                else:
                    nc.scalar.copy(ae[:, :], pa[:, :])
                aes[mi] = ae
            # G = A*cumK + E*cumC  (all-f16 DVE ops run in 2x mode)
            for r in range(nr):
                u = gwp.tile([128, TT], F16, tag="u", name="u")
                nc.vector.tensor_mul(u[:, :], aes[r][:, :], cums[r][:, tsl])
                v = gwp.tile([128, TT], F16, tag="v", name="v")
                nc.vector.tensor_mul(v[:, :], aes[nr + r][:, :], cums[nr + r][:, tsl])
                nc.vector.tensor_add(ghs[r][t][:, :], u[:, :], v[:, :])
            # software pipelining: finals trail by one chunk so the PE never
            # waits on the ACT/DVE chain that produces gh
            if prev_t is not None:
                emit_final(prev_t)
            prev_t = t
        emit_final(prev_t, last=True)

    nc.finalize()
    if hoist:
        split_excess_waits(nc)
    return nc


def _x_image(xc, nt, TT, nd):
    """[TC, D] f16 -> SBUF image [128, nt*nd*TT], t-major d-minor."""
    return np.ascontiguousarray(
        xc.reshape(nt, TT, nd, 128).transpose(3, 0, 2, 1).reshape(128, -1)
    )


def _w_image(w):
    """[C*128, W] -> SBUF image [128, C*W] (c-major blocks)."""
    c = w.shape[0] // 128
    return np.ascontiguousarray(
        w.reshape(c, 128, -1).transpose(1, 0, 2).reshape(128, -1)
    )


def make_core_inputs(x, attention_mask, PAE, PK, ZT, bvec):
    B, T, D = x.shape
    TC = T // N_SEQ_SHARDS
    R = ZT.shape[0]
    nq = (2 * R) // 128
    TT = 512
    nt, nd = TC // TT, D // 128
    m64 = np.asarray(attention_mask, np.float64)
    x32 = np.asarray(x, np.float32)
    n = np.cumsum(m64, axis=1)
    ninv = (1.0 / np.maximum(n, 1.0)).astype(np.float32)
    xs_full = (x32 * ninv[..., None]).astype(np.float16)
    all_ones = bool((m64 == 1.0).all())
    if all_ones:
        xk_full = x32.astype(np.float16)
    else:
        xk_full = (x32 * m64[..., None].astype(np.float32)).astype(np.float16)
    PAEi = _w_image(PAE.astype(np.float16))
    PKi = _w_image(PK.astype(np.float16))
    ZTi = _w_image(ZT.astype(np.float16))
    with_bias = bool(np.any(bvec))
    x64 = np.asarray(x, np.float64)

    in_maps = []
    for b in range(B):
        for h in range(N_SEQ_SHARDS):
            sl = slice(h * TC, (h + 1) * TC)
            if h == 0:
                S = np.zeros(2 * R, np.float64)
            else:
                xbar = (m64[b, :TC, None] * x64[b, :TC]).sum(0)
                S = xbar @ PK
            im = {
                "xsD": _x_image(xs_full[b, sl], nt, TT, nd),
                "xkD": _x_image(xk_full[b, sl], nt, TT, nd),
                "PAEd": PAEi,
                "PKd": PKi,
                "ZTd": ZTi,
                "initd": np.ascontiguousarray(
                    S.astype(np.float32).reshape(nq, 128).T
                ),
            }
            if with_bias:
                minn = np.minimum(n[b, sl], 1.0).astype(np.float16)
                im["minnd"] = np.ascontiguousarray(minn)[None, :]
                im["bvd"] = bvec.astype(np.float16)[None, :]
            in_maps.append(im)
    return in_maps


def unpack_out(arr, TC, D):
    """SBUF image [128, nt*nd*TT] -> [TC, D]."""
    TT = 512
    nt, nd = TC // TT, D // 128
    # arr[p, t*nd*TT + dd*TT + w] = out.T[dd*128+p, t*TT+w]
    outT = arr.reshape(128, nt, nd, TT).transpose(2, 0, 1, 3).reshape(D, TC)
    return outT.T


_NC_CACHE = {}


def get_nc(D, TC, R, with_bias=False):
    key = (D, TC, R, with_bias)
    if key not in _NC_CACHE:
        _NC_CACHE[key] = build_nc(D, TC, R, with_bias=with_bias)
    return _NC_CACHE[key]


def kernel(x, Wq, Wk, Wo, Winv, U, V, Wm, bias, alpha, attention_mask):
    x = np.asarray(x, np.float32)
    B, T, D = x.shape
    R = np.asarray(U).shape[1]
    TC = T // N_SEQ_SHARDS
    PAE, PK, ZT, bvec = fold_weights(Wq, Wk, Wo, Winv, U, V, Wm, bias, alpha)
    with_bias = bool(np.any(bvec))
    nc = get_nc(D, TC, R, with_bias)
    in_maps = make_core_inputs(x, np.asarray(attention_mask), PAE, PK, ZT, bvec)
    res = run_bass_kernel_spmd(nc, in_maps, core_ids=list(range(N_CORES)))
    out = np.empty((B, T, D), np.float32)
    k = 0
    for b in range(B):
        for h in range(N_SEQ_SHARDS):
            out[b, h * TC : (h + 1) * TC, :] = unpack_out(res.results[k]["outD"], TC, D)
            k += 1
    return out


# revision 7
# speedup vs baseline: 1.2537x; 1.2537x over previous
"""Trainium2 Bass kernel for nn_CausalBiBCNAttention (B=4, T=4096, D=1024, R=256).

Algebra (exact rewrite of the reference):
    out = G @ (Wo@U).T + min(n,1)*(1+alpha)*(Wo@bias)
    G   = (A*cumsum(Bk) + E*cumsum(C)) / max(n,1)
    A   = x @ (Wq.T V);  E = x @ (Wq.T Winv.T Wm)
    Bk  = (x @ (Wk.T Wm)) * m;  C = alpha * (x @ (Wk.T Winv.T V)) * m
    n   = cumsum(m)
The five DxD projections fold into four DxR matrices (host constant folding in
f64). Host-side prep folds the row scalings into the x streams:
    xs = x * (1/max(n,1))   (A/E stream -> G's division by n comes for free)
    xk = x * m              (K stream   -> masking comes for free)
so the device does only: 8 rank-128 projection groups per 512-col chunk,
native DVE prefix scans (f32 state, f16 out), two f16 multiplies + add for G,
and the final rank-R contraction with (Wo U).T. Everything streams fp16
single-pass (the 2e-2 harness gate leaves plenty of margin; measured ~5e-4).

All tensors are staged host-side in the exact SBUF image layout (2D
contiguous, 128 descriptors/DMA) and the transfers are spread across the
three DMA queues (SP + ACT hardware DGE, Pool software DGE) so input
streaming never gates the PE.

Sharding: 8 cores = batch(4) x sequence-halves(2). The cumsum carry S for the
second half and the 1/n rows are computed on the host (cheap O(B*T*D) numpy)
and passed as tiny inputs, so no cross-core or xprev streaming is needed.
"""

from contextlib import ExitStack

import numpy as np

import concourse.bass as bass
import concourse.mybir as mybir
import concourse.tile as tile
from concourse.bass_utils import run_bass_kernel_spmd

F32 = mybir.dt.float32
F16 = mybir.dt.float16
AL = mybir.AluOpType

N_CORES = 8
N_SEQ_SHARDS = 2


def fold_weights(Wq, Wk, Wo, Winv, U, V, Wm, bias, alpha):
    Wq, Wk, Wo, Winv, U, V, Wm, bias = (
        np.asarray(a, np.float64) for a in (Wq, Wk, Wo, Winv, U, V, Wm, bias)
    )
    alpha = float(alpha)
    P1 = Wq.T @ V
    P2 = Wq.T @ Winv.T @ Wm
    P3 = Wk.T @ Wm
    P4 = alpha * (Wk.T @ (Winv.T @ V))
    PAE = np.concatenate([P1, P2], axis=1)          # [D, 2R] f64
    PK = np.concatenate([P3, P4], axis=1)           # [D, 2R] f64
    ZT = np.ascontiguousarray((Wo @ U).T)           # [R, D] f64
    bvec = ((1.0 + alpha) * (Wo @ bias))            # [D] f64
    return PAE, PK, ZT, bvec


def split_excess_waits(nc, max_waits=1):
    """Hoist excess per-instruction sync waits onto preceding same-engine NoOps.

    Walrus's per-instruction sync budget rejects >1 wait command on several
    instruction structs (fp32 Matmult, DMA pseudo-ops). Engine streams execute
    in order, so a NoOp carrying the extra wait immediately before the
    instruction is semantically identical.
    """
    fn = nc.m.functions[0]
    k = 0
    for blk in fn.blocks:
        new_insts = []
        for ins in blk.instructions:
            si = getattr(ins, "sync_info", None)
            if si is not None and si.on_wait and len(si.on_wait) > max_waits:
                waits = list(si.on_wait)
                for w in waits[:-max_waits]:
                    k += 1
                    new_insts.append(
                        mybir.InstNoOp(
                            name=f"{ins.name}-hoistw{k}",
                            engine=ins.engine,
                            ins=[],
                            outs=[],
                            sync_info=mybir.SyncInfo(on_wait=[w], on_update=[]),
                            bass_nofuse=True,
                        )
                    )
                ins.sync_info = mybir.SyncInfo(
                    on_wait=waits[-max_waits:], on_update=si.on_update
                )
            new_insts.append(ins)
        blk.instructions[:] = new_insts
    return nc


def build_nc(D, TC, R, TT=512, with_bias=False, hoist=True):
    assert D % 128 == 0 and R % 128 == 0 and TC % TT == 0
    nd, nr, nt = D // 128, R // 128, TC // TT
    nq = 2 * nr            # cumsum streams: [Bk ranks | C ranks]
    W2 = 2 * R             # projection width per stream pair
    XW = nd * TT           # x-stream image columns per t-chunk

    nc = bass.Bass()
    # all inputs are pre-staged SBUF images: [128, cols], plain 2D DMAs
    xsD = nc.dram_tensor("xsD", (128, nt * XW), F16, kind="ExternalInput")
    xkD = nc.dram_tensor("xkD", (128, nt * XW), F16, kind="ExternalInput")
    PAEd = nc.dram_tensor("PAEd", (128, nd * W2), F16, kind="ExternalInput")
    PKd = nc.dram_tensor("PKd", (128, nd * W2), F16, kind="ExternalInput")
    ZTd = nc.dram_tensor("ZTd", (128, nr * D), F16, kind="ExternalInput")
    initd = nc.dram_tensor("initd", (128, nq), F32, kind="ExternalInput")
    if with_bias:
        minnd = nc.dram_tensor("minnd", (1, TC), F16, kind="ExternalInput")
        bvd = nc.dram_tensor("bvd", (1, D), F16, kind="ExternalInput")
    outD = nc.dram_tensor("outD", (128, nt * XW), F16, kind="ExternalOutput")

    with tile.TileContext(nc) as tc, ExitStack() as ctx:
        res = ctx.enter_context(tc.tile_pool(name="res", bufs=1))
        psb = ctx.enter_context(tc.tile_pool(name="psb", bufs=8, space="PSUM"))
        aep = ctx.enter_context(tc.tile_pool(name="aep", bufs=6))
        gwp = ctx.enter_context(tc.tile_pool(name="gwp", bufs=4))
        otp = ctx.enter_context(tc.tile_pool(name="otp", bufs=2))

        # resident tiles; x streams are t-major, d-minor column blocks
        xk = res.tile([128, nt * XW], F16, tag="xk", name="xk")
        xs = res.tile([128, nt * XW], F16, tag="xs", name="xs")
        pk = res.tile([128, nd * W2], F16, tag="pk", name="pk")
        pae = res.tile([128, nd * W2], F16, tag="pae", name="pae")
        zt = res.tile([128, nr * D], F16, tag="zt", name="zt")
        cums = [
            res.tile([128, TC], F16, tag=f"cum{q}", name=f"cum{q}")
            for q in range(nq)
        ]
        ghs = [
            [
                res.tile([128, TT], F16, tag=f"gh{r}_{t}", name=f"gh{r}_{t}")
                for t in range(nt)
            ]
            for r in range(nr)
        ]
        initt = res.tile([128, nq], F32, tag="initt", name="initt")
        zdum = res.tile([128, TT], F16, tag="zdum", name="zdum")
        if with_bias:
            minnt = res.tile([1, TC], F16, tag="minnt", name="minnt")
            bvt = res.tile([1, D], F16, tag="bvt", name="bvt")

        nc.vector.memset(zdum[:, :], 0.0)

        # DMA queue assignment. The Pool SWDGE queue is ~2.5x slower than the
        # two HWDGE queues (SP, ACT), so both x streams interleave on SP and
        # the weights ride ACT; Pool only carries mid-kernel output chunks,
        # which have big slack. pk/xk0 are split in d-halves so the first K
        # matmuls start on the first half.
        HW2 = nd * W2 // 2
        nc.sync.dma_start(xk[:, 0 : XW // 2], xkD[:, 0 : XW // 2])
        nc.scalar.dma_start(pk[:, 0:HW2], PKd[:, 0:HW2])
        nc.sync.dma_start(xk[:, XW // 2 : XW], xkD[:, XW // 2 : XW])
        nc.scalar.dma_start(pk[:, HW2:], PKd[:, HW2:])
        nc.scalar.dma_start(initt[:, :], initd[:, :])
        nc.sync.dma_start(xs[:, 0:XW], xsD[:, 0:XW])
        nc.scalar.dma_start(pae[:, :], PAEd[:, :])
        for t in range(1, nt):
            nc.sync.dma_start(xk[:, t * XW : (t + 1) * XW], xkD[:, t * XW : (t + 1) * XW])
            nc.sync.dma_start(xs[:, t * XW : (t + 1) * XW], xsD[:, t * XW : (t + 1) * XW])
        nc.scalar.dma_start(zt[:, :], ZTd[:, :])
        if with_bias:
            nc.scalar.dma_start(minnt[:, :], minnd[:, :])
            nc.scalar.dma_start(bvt[:, :], bvd[:, :])

        def emit_final(t, last=False):
            tsl = slice(t * TT, (t + 1) * TT)
            ot = otp.tile([128, XW], F16, tag="ot", name="ot")
            for dd in range(nd):
                po = psb.tile([128, TT], F32, tag="pt", name="pt")
                for r in range(nr):
                    nc.tensor.matmul(
                        po[:, :],
                        zt[:, r * D + dd * 128 : r * D + (dd + 1) * 128],
                        ghs[r][t][:, :],
                        start=(r == 0),
                        stop=(r == nr - 1 and not with_bias),
                    )
                if with_bias:
                    nc.tensor.matmul(
                        po[:, :],
                        bvt[0:1, dd * 128 : (dd + 1) * 128],
                        minnt[0:1, tsl],
                        start=False,
                        stop=True,
                    )
                osl = slice(dd * TT, (dd + 1) * TT)
                # alternate evacuation engines: a single ACT can't drain PSUM
                # as fast as the PE fills it in the final phase
                if dd % 2 == 1:
                    nc.vector.tensor_copy(ot[:, osl], po[:, :])
                else:
                    nc.scalar.copy(ot[:, osl], po[:, :])
                if last and dd == nd // 2 - 1:
                    nc.scalar.dma_start(
                        outD[:, t * XW : t * XW + XW // 2], ot[:, 0 : XW // 2]
                    )
            if last:
                nc.sync.dma_start(
                    outD[:, t * XW + XW // 2 : (t + 1) * XW], ot[:, XW // 2 :]
                )
            else:
                nc.gpsimd.dma_start(outD[:, t * XW : (t + 1) * XW], ot[:, :])

        # A/E stream order pairs (A_r, E_r) adjacently so gh[r] can start as
        # soon as its two operands are evacuated
        m_order = []
        for r in range(nr):
            m_order += [r, nr + r]

        prev_t = None
        for t in range(nt):
            tsl = slice(t * TT, (t + 1) * TT)
            xoff = t * XW
            last = t == nt - 1
            # K-side projections -> prefix scans (f32 state, f16 out)
            for q in range(nq):
                pt = psb.tile([128, TT], F32, tag="pt", name="pt")
                for dd in range(nd):
                    nc.tensor.matmul(
                        pt[:, :],
                        pk[:, dd * W2 + q * 128 : dd * W2 + (q + 1) * 128],
                        xk[:, xoff + dd * TT : xoff + (dd + 1) * TT],
                        start=(dd == 0),
                        stop=(dd == nd - 1),
                    )
                init = initt[:, q : q + 1] if t == 0 else cums[q][:, t * TT - 1 : t * TT]
                nc.vector.tensor_tensor_scan(
                    cums[q][:, tsl], pt[:, :], zdum[:, :], init, AL.add, AL.bypass
                )
            # A/E projections (xs carries the 1/n row scaling)
            aes = [None] * nq
            for k, mi in enumerate(m_order):
                pa = psb.tile([128, TT], F32, tag="pt", name="pt")
                for dd in range(nd):
                    nc.tensor.matmul(
                        pa[:, :],
                        pae[:, dd * W2 + mi * 128 : dd * W2 + (mi + 1) * 128],
                        xs[:, xoff + dd * TT : xoff + (dd + 1) * TT],
                        start=(dd == 0),
                        stop=(dd == nd - 1),
                    )
                ae = aep.tile([128, TT], F16, tag="ae", name="ae")
                if last and k % 2 == 1:
                    nc.vector.tensor_copy(ae[:, :], pa[:, :])
                else:
                    nc.scalar.copy(ae[:, :], pa[:, :])
                aes[mi] = ae
            # G = A*cumK + E*cumC  (all-f16 DVE ops run in 2x mode)
            for r in range(nr):
                u = gwp.tile([128, TT], F16, tag="u", name="u")
                nc.vector.tensor_mul(u[:, :], aes[r][:, :], cums[r][:, tsl])
                v = gwp.tile([128, TT], F16, tag="v", name="v")
                nc.vector.tensor_mul(v[:, :], aes[nr + r][:, :], cums[nr + r][:, tsl])
                nc.vector.tensor_add(ghs[r][t][:, :], u[:, :], v[:, :])
            # software pipelining: finals trail by one chunk so the PE never
            # waits on the ACT/DVE chain that produces gh
            if prev_t is not None:
                emit_final(prev_t)
            prev_t = t
        emit_final(prev_t, last=True)

    nc.finalize()
    if hoist:
        split_excess_waits(nc)
    return nc


def _x_image(xc, nt, TT, nd):
    """[TC, D] f16 -> SBUF image [128, nt*nd*TT], t-major d-minor."""
    return np.ascontiguousarray(
        xc.reshape(nt, TT, nd, 128).transpose(3, 0, 2, 1).reshape(128, -1)
    )


def _w_image(w):
    """[C*128, W] -> SBUF image [128, C*W] (c-major blocks)."""
    c = w.shape[0] // 128
    return np.ascontiguousarray(
        w.reshape(c, 128, -1).transpose(1, 0, 2).reshape(128, -1)
    )


def make_core_inputs(x, attention_mask, PAE, PK, ZT, bvec):
    B, T, D = x.shape
    TC = T // N_SEQ_SHARDS
    R = ZT.shape[0]
    nq = (2 * R) // 128
    TT = 512
    nt, nd = TC // TT, D // 128
    m64 = np.asarray(attention_mask, np.float64)
    x32 = np.asarray(x, np.float32)
    n = np.cumsum(m64, axis=1)
    ninv = (1.0 / np.maximum(n, 1.0)).astype(np.float32)
    xs_full = (x32 * ninv[..., None]).astype(np.float16)
    all_ones = bool((m64 == 1.0).all())
    if all_ones:
        xk_full = x32.astype(np.float16)
    else:
        xk_full = (x32 * m64[..., None].astype(np.float32)).astype(np.float16)
    PAEi = _w_image(PAE.astype(np.float16))
    PKi = _w_image(PK.astype(np.float16))
    ZTi = _w_image(ZT.astype(np.float16))
    with_bias = bool(np.any(bvec))
    x64 = np.asarray(x, np.float64)

    in_maps = []
    for b in range(B):
        for h in range(N_SEQ_SHARDS):
            sl = slice(h * TC, (h + 1) * TC)
            if h == 0:
                S = np.zeros(2 * R, np.float64)
            else:
                xbar = (m64[b, :TC, None] * x64[b, :TC]).sum(0)
                S = xbar @ PK
            im = {
                "xsD": _x_image(xs_full[b, sl], nt, TT, nd),
                "xkD": _x_image(xk_full[b, sl], nt, TT, nd),
                "PAEd": PAEi,
                "PKd": PKi,
                "ZTd": ZTi,
                "initd": np.ascontiguousarray(
                    S.astype(np.float32).reshape(nq, 128).T
                ),
            }
            if with_bias:
                minn = np.minimum(n[b, sl], 1.0).astype(np.float16)
                im["minnd"] = np.ascontiguousarray(minn)[None, :]
                im["bvd"] = bvec.astype(np.float16)[None, :]
            in_maps.append(im)
    return in_maps


def unpack_out(arr, TC, D):
    """SBUF image [128, nt*nd*TT] -> [TC, D]."""
    TT = 512
    nt, nd = TC // TT, D // 128
    # arr[p, t*nd*TT + dd*TT + w] = out.T[dd*128+p, t*TT+w]
    outT = arr.reshape(128, nt, nd, TT).transpose(2, 0, 1, 3).reshape(D, TC)
    return outT.T


_NC_CACHE = {}


def get_nc(D, TC, R, with_bias=False):
    key = (D, TC, R, with_bias)
    if key not in _NC_CACHE:
        _NC_CACHE[key] = build_nc(D, TC, R, with_bias=with_bias)
    return _NC_CACHE[key]


def kernel(x, Wq, Wk, Wo, Winv, U, V, Wm, bias, alpha, attention_mask):
    x = np.asarray(x, np.float32)
    B, T, D = x.shape
    R = np.asarray(U).shape[1]
    TC = T // N_SEQ_SHARDS
    PAE, PK, ZT, bvec = fold_weights(Wq, Wk, Wo, Winv, U, V, Wm, bias, alpha)
    with_bias = bool(np.any(bvec))
    nc = get_nc(D, TC, R, with_bias)
    in_maps = make_core_inputs(x, np.asarray(attention_mask), PAE, PK, ZT, bvec)
    res = run_bass_kernel_spmd(nc, in_maps, core_ids=list(range(N_CORES)))
    out = np.empty((B, T, D), np.float32)
    k = 0
    for b in range(B):
        for h in range(N_SEQ_SHARDS):
            out[b, h * TC : (h + 1) * TC, :] = unpack_out(res.results[k]["outD"], TC, D)
            k += 1
    return out


# revision 8
# speedup vs baseline: 1.2924x; 1.0309x over previous
"""Trainium2 Bass kernel for nn_CausalBiBCNAttention (B=4, T=4096, D=1024, R=256).

Algebra (exact rewrite of the reference):
    out = G @ (Wo@U).T + min(n,1)*(1+alpha)*(Wo@bias)
    G   = (A*cumsum(Bk) + E*cumsum(C)) / max(n,1)
    A   = x @ (Wq.T V);  E = x @ (Wq.T Winv.T Wm)
    Bk  = (x @ (Wk.T Wm)) * m;  C = alpha * (x @ (Wk.T Winv.T V)) * m
    n   = cumsum(m)
The five DxD projections fold into four DxR matrices (host constant folding in
f64). Host-side prep folds the row scalings into the x streams:
    xs = x * (1/max(n,1))   (A/E stream -> G's division by n comes for free)
    xk = x * m              (K stream   -> masking comes for free)
so the device does only: 8 rank-128 projection groups per column chunk,
native DVE prefix scans (f32 state, f16 out), two f16 multiplies + add for G,
and the final rank-R contraction with (Wo U).T. Everything streams fp16
single-pass (the 2e-2 harness gate leaves plenty of margin; measured ~5e-4).

Column chunks are non-uniform (256, 512, 512, 512, 256): a narrow first chunk
starts the pipeline on less DMA'd data, and a narrow last chunk halves the
serial drain tail (evac -> G -> final matmul -> output DMA).

All tensors are staged host-side in the exact SBUF image layout (2D
contiguous, 128 descriptors/DMA). Both x streams ride the SP hardware-DGE
queue, weights ride the ACT hardware-DGE queue, and mid-kernel outputs (which
have slack) ride them round-robin; the slow Pool software-DGE queue is unused.

Sharding: 8 cores = batch(4) x sequence-halves(2). The cumsum carry S for the
second half and the 1/n rows are computed on the host (cheap O(B*T*D) numpy)
and passed as tiny inputs, so no cross-core or xprev streaming is needed.
"""

from contextlib import ExitStack

import numpy as np

import concourse.bass as bass
import concourse.mybir as mybir
import concourse.tile as tile
from concourse.bass_utils import run_bass_kernel_spmd

F32 = mybir.dt.float32
F16 = mybir.dt.float16
AL = mybir.AluOpType

N_CORES = 8
N_SEQ_SHARDS = 2


def chunk_widths(TC):
    if TC >= 1536 and (TC - 512) % 512 == 0:
        return [256] + [512] * ((TC - 512) // 512) + [256]
    assert TC % 512 == 0
    return [512] * (TC // 512)


def fold_weights(Wq, Wk, Wo, Winv, U, V, Wm, bias, alpha):
    Wq, Wk, Wo, Winv, U, V, Wm, bias = (
        np.asarray(a, np.float64) for a in (Wq, Wk, Wo, Winv, U, V, Wm, bias)
    )
    alpha = float(alpha)
    P1 = Wq.T @ V
    P2 = Wq.T @ Winv.T @ Wm
    P3 = Wk.T @ Wm
    P4 = alpha * (Wk.T @ (Winv.T @ V))
    PAE = np.concatenate([P1, P2], axis=1)          # [D, 2R] f64
    PK = np.concatenate([P3, P4], axis=1)           # [D, 2R] f64
    ZT = np.ascontiguousarray((Wo @ U).T)           # [R, D] f64
    bvec = ((1.0 + alpha) * (Wo @ bias))            # [D] f64
    return PAE, PK, ZT, bvec


def split_excess_waits(nc, max_waits=1):
    """Hoist excess per-instruction sync waits onto preceding same-engine NoOps.

    Walrus's per-instruction sync budget rejects >1 wait command on several
    instruction structs (fp32 Matmult, DMA pseudo-ops). Engine streams execute
    in order, so a NoOp carrying the extra wait immediately before the
    instruction is semantically identical.
    """
    fn = nc.m.functions[0]
    k = 0
    for blk in fn.blocks:
        new_insts = []
        for ins in blk.instructions:
            si = getattr(ins, "sync_info", None)
            if si is not None and si.on_wait and len(si.on_wait) > max_waits:
                waits = list(si.on_wait)
                for w in waits[:-max_waits]:
                    k += 1
                    new_insts.append(
                        mybir.InstNoOp(
                            name=f"{ins.name}-hoistw{k}",
                            engine=ins.engine,
                            ins=[],
                            outs=[],
                            sync_info=mybir.SyncInfo(on_wait=[w], on_update=[]),
                            bass_nofuse=True,
                        )
                    )
                ins.sync_info = mybir.SyncInfo(
                    on_wait=waits[-max_waits:], on_update=si.on_update
                )
            new_insts.append(ins)
        blk.instructions[:] = new_insts
    return nc


def build_nc(D, TC, R, with_bias=False, hoist=True):
    assert D % 128 == 0 and R % 128 == 0
    nd, nr = D // 128, R // 128
    nq = 2 * nr            # cumsum streams: [Bk ranks | C ranks]
    W2 = 2 * R             # projection width per stream pair
    cs = chunk_widths(TC)
    nt = len(cs)
    toff = [sum(cs[:i]) for i in range(nt + 1)]           # offsets in T cols
    ioff = [nd * o for o in toff]                          # offsets in image cols
    XWT = nd * TC                                          # total image cols
    CMAX = max(cs)

    nc = bass.Bass()
    # all inputs are pre-staged SBUF images: [128, cols], plain 2D DMAs
    xsD = nc.dram_tensor("xsD", (128, XWT), F16, kind="ExternalInput")
    xkD = nc.dram_tensor("xkD", (128, XWT), F16, kind="ExternalInput")
    PAEd = nc.dram_tensor("PAEd", (128, nd * W2), F16, kind="ExternalInput")
    PKd = nc.dram_tensor("PKd", (128, nd * W2), F16, kind="ExternalInput")
    ZTd = nc.dram_tensor("ZTd", (128, nr * D), F16, kind="ExternalInput")
    initd = nc.dram_tensor("initd", (128, nq), F32, kind="ExternalInput")
    if with_bias:
        minnd = nc.dram_tensor("minnd", (1, TC), F16, kind="ExternalInput")
        bvd = nc.dram_tensor("bvd", (1, D), F16, kind="ExternalInput")
    outD = nc.dram_tensor("outD", (128, XWT), F16, kind="ExternalOutput")

    with tile.TileContext(nc) as tc, ExitStack() as ctx:
        res = ctx.enter_context(tc.tile_pool(name="res", bufs=1))
        psb = ctx.enter_context(tc.tile_pool(name="psb", bufs=8, space="PSUM"))
        aep = ctx.enter_context(tc.tile_pool(name="aep", bufs=6))
        gwp = ctx.enter_context(tc.tile_pool(name="gwp", bufs=4))
        otp = ctx.enter_context(tc.tile_pool(name="otp", bufs=2))

        # resident tiles; x streams are chunk-major, d-minor column blocks
        xk = res.tile([128, XWT], F16, tag="xk", name="xk")
        xs = res.tile([128, XWT], F16, tag="xs", name="xs")
        pk = res.tile([128, nd * W2], F16, tag="pk", name="pk")
        pae = res.tile([128, nd * W2], F16, tag="pae", name="pae")
        zt = res.tile([128, nr * D], F16, tag="zt", name="zt")
        cums = [
            res.tile([128, TC], F16, tag=f"cum{q}", name=f"cum{q}")
            for q in range(nq)
        ]
        ghs = [
            [
                res.tile([128, cs[t]], F16, tag=f"gh{r}_{t}", name=f"gh{r}_{t}")
                for t in range(nt)
            ]
            for r in range(nr)
        ]
        initt = res.tile([128, nq], F32, tag="initt", name="initt")
        zdum = res.tile([128, CMAX], F16, tag="zdum", name="zdum")
        if with_bias:
            minnt = res.tile([1, TC], F16, tag="minnt", name="minnt")
            bvt = res.tile([1, D], F16, tag="bvt", name="bvt")

        nc.vector.memset(zdum[:, :], 0.0)

        # DMA queue assignment. The Pool SWDGE queue is slow and lazy, so
        # everything rides the two HWDGE queues: both x streams interleave on
        # SP, weights on ACT. pk and xk chunk 0 are split in halves so the
        # first K matmuls start sooner. Outputs round-robin on both queues.
        HW2 = nd * W2 // 2
        X0h = ioff[1] // 2
        nc.sync.dma_start(xk[:, 0:X0h], xkD[:, 0:X0h])
        nc.scalar.dma_start(pk[:, 0:HW2], PKd[:, 0:HW2])
        nc.sync.dma_start(xk[:, X0h : ioff[1]], xkD[:, X0h : ioff[1]])
        nc.scalar.dma_start(pk[:, HW2:], PKd[:, HW2:])
        nc.scalar.dma_start(initt[:, :], initd[:, :])
        nc.sync.dma_start(xs[:, 0 : ioff[1]], xsD[:, 0 : ioff[1]])
        nc.scalar.dma_start(pae[:, :], PAEd[:, :])
        for t in range(1, nt):
            nc.sync.dma_start(xk[:, ioff[t] : ioff[t + 1]], xkD[:, ioff[t] : ioff[t + 1]])
            nc.sync.dma_start(xs[:, ioff[t] : ioff[t + 1]], xsD[:, ioff[t] : ioff[t + 1]])
        nc.scalar.dma_start(zt[:, :], ZTd[:, :])
        if with_bias:
            nc.scalar.dma_start(minnt[:, :], minnd[:, :])
            nc.scalar.dma_start(bvt[:, :], bvd[:, :])

        def emit_final(t):
            ct = cs[t]
            last = t == nt - 1
            ot = otp.tile([128, nd * CMAX], F16, tag="ot", name="ot")
            for dd in range(nd):
                po = psb.tile([128, CMAX], F32, tag="pt", name="pt")
                for r in range(nr):
                    nc.tensor.matmul(
                        po[:, 0:ct],
                        zt[:, r * D + dd * 128 : r * D + (dd + 1) * 128],
                        ghs[r][t][:, :],
                        start=(r == 0),
                        stop=(r == nr - 1 and not with_bias),
                    )
                if with_bias:
                    nc.tensor.matmul(
                        po[:, 0:ct],
                        bvt[0:1, dd * 128 : (dd + 1) * 128],
                        minnt[0:1, toff[t] : toff[t + 1]],
                        start=False,
                        stop=True,
                    )
                osl = slice(dd * ct, (dd + 1) * ct)
                # alternate evacuation engines: a single ACT can't drain PSUM
                # as fast as the PE fills it in the final phase
                if dd % 2 == 1:
                    nc.vector.tensor_copy(ot[:, osl], po[:, 0:ct])
                else:
                    nc.scalar.copy(ot[:, osl], po[:, 0:ct])
                if last and dd == nd // 2 - 1:
                    nc.scalar.dma_start(
                        outD[:, ioff[t] : ioff[t] + nd * ct // 2],
                        ot[:, 0 : nd * ct // 2],
                    )
            if last:
                nc.sync.dma_start(
                    outD[:, ioff[t] + nd * ct // 2 : ioff[t + 1]],
                    ot[:, nd * ct // 2 : nd * ct],
                )
            elif t % 2 == 0:
                nc.sync.dma_start(outD[:, ioff[t] : ioff[t + 1]], ot[:, 0 : nd * ct])
            else:
                nc.scalar.dma_start(outD[:, ioff[t] : ioff[t + 1]], ot[:, 0 : nd * ct])

        # A/E stream order pairs (A_r, E_r) adjacently so gh[r] can start as
        # soon as its two operands are evacuated
        m_order = []
        for r in range(nr):
            m_order += [r, nr + r]

        prev_t = None
        for t in range(nt):
            ct = cs[t]
            tsl = slice(toff[t], toff[t + 1])
            xo = ioff[t]
            # K-side projections -> prefix scans (f32 state, f16 out)
            for q in range(nq):
                pt = psb.tile([128, CMAX], F32, tag="pt", name="pt")
                for dd in range(nd):
                    nc.tensor.matmul(
                        pt[:, 0:ct],
                        pk[:, dd * W2 + q * 128 : dd * W2 + (q + 1) * 128],
                        xk[:, xo + dd * ct : xo + (dd + 1) * ct],
                        start=(dd == 0),
                        stop=(dd == nd - 1),
                    )
                init = (
                    initt[:, q : q + 1]
                    if t == 0
                    else cums[q][:, toff[t] - 1 : toff[t]]
                )
                nc.vector.tensor_tensor_scan(
                    cums[q][:, tsl], pt[:, 0:ct], zdum[:, 0:ct], init, AL.add, AL.bypass
                )
            # A/E projections (xs carries the 1/n row scaling)
            aes = [None] * nq
            for k, mi in enumerate(m_order):
                pa = psb.tile([128, CMAX], F32, tag="pt", name="pt")
                for dd in range(nd):
                    nc.tensor.matmul(
                        pa[:, 0:ct],
                        pae[:, dd * W2 + mi * 128 : dd * W2 + (mi + 1) * 128],
                        xs[:, xo + dd * ct : xo + (dd + 1) * ct],
                        start=(dd == 0),
                        stop=(dd == nd - 1),
                    )
                ae = aep.tile([128, CMAX], F16, tag="ae", name="ae")
                if k % 2 == 1:
                    nc.vector.tensor_copy(ae[:, 0:ct], pa[:, 0:ct])
                else:
                    nc.scalar.copy(ae[:, 0:ct], pa[:, 0:ct])
                aes[mi] = ae
            # G = A*cumK + E*cumC  (all-f16 DVE ops run in 2x mode)
            for r in range(nr):
                u = gwp.tile([128, CMAX], F16, tag="u", name="u")
                nc.vector.tensor_mul(u[:, 0:ct], aes[r][:, 0:ct], cums[r][:, tsl])
                v = gwp.tile([128, CMAX], F16, tag="v", name="v")
                nc.vector.tensor_mul(v[:, 0:ct], aes[nr + r][:, 0:ct], cums[nr + r][:, tsl])
                nc.vector.tensor_add(ghs[r][t][:, :], u[:, 0:ct], v[:, 0:ct])
            # software pipelining: finals trail by one chunk so the PE never
            # waits on the ACT/DVE chain that produces gh
            if prev_t is not None:
                emit_final(prev_t)
            prev_t = t
        emit_final(prev_t)

    nc.finalize()
    if hoist:
        split_excess_waits(nc)
    return nc


def _x_image(xc, cs, nd):
    """[TC, D] f16 -> SBUF image [128, nd*TC], chunk-major d-minor."""
    blocks = []
    o = 0
    for ct in cs:
        b = xc[o : o + ct]                      # [ct, D]
        blocks.append(b.reshape(ct, nd, 128).transpose(2, 1, 0).reshape(128, -1))
        o += ct
    return np.ascontiguousarray(np.concatenate(blocks, axis=1))


def _w_image(w):
    """[C*128, W] -> SBUF image [128, C*W] (c-major blocks)."""
    c = w.shape[0] // 128
    return np.ascontiguousarray(
        w.reshape(c, 128, -1).transpose(1, 0, 2).reshape(128, -1)
    )


def make_core_inputs(x, attention_mask, PAE, PK, ZT, bvec):
    B, T, D = x.shape
    TC = T // N_SEQ_SHARDS
    R = ZT.shape[0]
    nq = (2 * R) // 128
    nd = D // 128
    cs = chunk_widths(TC)
    m64 = np.asarray(attention_mask, np.float64)
    x32 = np.asarray(x, np.float32)
    n = np.cumsum(m64, axis=1)
    ninv = (1.0 / np.maximum(n, 1.0)).astype(np.float32)
    xs_full = (x32 * ninv[..., None]).astype(np.float16)
    all_ones = bool((m64 == 1.0).all())
    if all_ones:
        xk_full = x32.astype(np.float16)
    else:
        xk_full = (x32 * m64[..., None].astype(np.float32)).astype(np.float16)
    PAEi = _w_image(PAE.astype(np.float16))
    PKi = _w_image(PK.astype(np.float16))
    ZTi = _w_image(ZT.astype(np.float16))
    with_bias = bool(np.any(bvec))
    x64 = np.asarray(x, np.float64)

    in_maps = []
    for b in range(B):
        for h in range(N_SEQ_SHARDS):
            sl = slice(h * TC, (h + 1) * TC)
            if h == 0:
                S = np.zeros(2 * R, np.float64)
            else:
                xbar = (m64[b, :TC, None] * x64[b, :TC]).sum(0)
                S = xbar @ PK
            im = {
                "xsD": _x_image(xs_full[b, sl], cs, nd),
                "xkD": _x_image(xk_full[b, sl], cs, nd),
                "PAEd": PAEi,
                "PKd": PKi,
                "ZTd": ZTi,
                "initd": np.ascontiguousarray(
                    S.astype(np.float32).reshape(nq, 128).T
                ),
            }
            if with_bias:
                minn = np.minimum(n[b, sl], 1.0).astype(np.float16)
                im["minnd"] = np.ascontiguousarray(minn)[None, :]
                im["bvd"] = bvec.astype(np.float16)[None, :]
            in_maps.append(im)
    return in_maps


def unpack_out(arr, TC, D):
    """SBUF image [128, nd*TC] (chunk-major d-minor) -> [TC, D]."""
    nd = D // 128
    cs = chunk_widths(TC)
    out = np.empty((TC, D), arr.dtype)
    o = 0
    for ct in cs:
        blk = arr[:, nd * o : nd * (o + ct)].reshape(128, nd, ct)
        out[o : o + ct] = blk.transpose(2, 1, 0).reshape(ct, D)
        o += ct
    return out


_NC_CACHE = {}


def get_nc(D, TC, R, with_bias=False):
    key = (D, TC, R, with_bias)
    if key not in _NC_CACHE:
        _NC_CACHE[key] = build_nc(D, TC, R, with_bias=with_bias)
    return _NC_CACHE[key]


def kernel(x, Wq, Wk, Wo, Winv, U, V, Wm, bias, alpha, attention_mask):
    x = np.asarray(x, np.float32)
    B, T, D = x.shape
    R = np.asarray(U).shape[1]
    TC = T // N_SEQ_SHARDS
    PAE, PK, ZT, bvec = fold_weights(Wq, Wk, Wo, Winv, U, V, Wm, bias, alpha)
    with_bias = bool(np.any(bvec))
    nc = get_nc(D, TC, R, with_bias)
    in_maps = make_core_inputs(x, np.asarray(attention_mask), PAE, PK, ZT, bvec)
    res = run_bass_kernel_spmd(nc, in_maps, core_ids=list(range(N_CORES)))
    out = np.empty((B, T, D), np.float32)
    k = 0
    for b in range(B):
        for h in range(N_SEQ_SHARDS):
            out[b, h * TC : (h + 1) * TC, :] = unpack_out(res.results[k]["outD"], TC, D)
            k += 1
    return out
